# revision 18
# baseline (speedup 1.0000x reference)
"""Trainium2 Bass kernel for the NP/NY/NU RNN scan (nn_BlackBoxModel_24489903521937).

Model (per step t, batch row b):
    x_t   = [y_t, y_{t-4..t-1}, u_{t-4..t-1}, u_t]          (60)
    h1    = tanh(x_t @ W1 + b1)                              (128)
    h2    = tanh(h1 @ W2 + b2)                               (128)
    y_{t+1} = h2 @ W3 + b3                                   (8)
    output ys[:, t] = y_t

Strategy (pure data parallel, batch 4096 -> 8 cores x 512):
  * feature-major layout: features on SBUF partitions, batch on the free dim.
  * y-history lives in 4 ring slots of a [128, B] staging tile, one slot per
    32-partition strip (SBUF APs must start at partition 0/32/64/96).  The
    x @ W1 product becomes: one K=128 matmul against phase-permuted W1 blocks
    (C_p, zero rows where a slot is semantically dead), one K=20 sliding
    u-window matmul, and one composed (W3 @ A0) matmul from h2 directly, so
    the recurrent cycle is just tanh -> mm(W2) -> tanh -> mm(W3 A0).
  * y_{t-4} is read from the slot y_t is about to overwrite: emission order
    (mmX before the staging write) makes Tile sequence the write after the
    read, so no extra buffering is needed.
  * outputs retire from the staging tile by raw feature-major DMA every 4
    steps; the host does the final [T,8,B] -> [B,T,8] transpose.
  * matmul operands are fp16 (1 cycle/row, fp32 PSUM accumulate); the
    5-step fading memory of the state keeps fp16 error flat (~6e-4).
"""

import numpy as np

NP_, NY, NU = 4, 8, 4
B, T, H = 4096, 256, 128
NCORES = 8
BC = B // NCORES  # 512 batch rows per core
CHUNKS = 2        # column chunks for the critical tanh/matmul cycle
CW = BC // CHUNKS
PF = 6            # u-window DMA prefetch depth (steps ahead)
NSLOT = 4         # y ring slots (one per 32-partition strip)

_COMPILED = {}


def _build_program():
    import concourse.mybir as mybir
    import concourse.tile as tile
    from concourse import bacc

    f32 = mybir.dt.float32
    fh = mybir.dt.float16
    Tanh = mybir.ActivationFunctionType.Tanh

    nc = bacc.Bacc("TRN2", target_bir_lowering=False, debug=False)

    d_stag0 = nc.dram_tensor("stag0", [128, BC], fh, kind="ExternalInput")
    d_uwin = nc.dram_tensor("uwin", [T, 20, BC], fh, kind="ExternalInput")
    d_bs = nc.dram_tensor("bs", [20, 128], fh, kind="ExternalInput")
    # 8 C matrices: [0..3] steady phases (t % 4), [4..7] boot steps t=0..3
    d_cmats = nc.dram_tensor("cmats", [128, 8 * 128], fh, kind="ExternalInput")
    d_w2 = nc.dram_tensor("w2", [128, 128], fh, kind="ExternalInput")
    d_wc = nc.dram_tensor("wc", [128, 128], fh, kind="ExternalInput")
    d_w3 = nc.dram_tensor("w3", [128, 8], fh, kind="ExternalInput")
    d_b1 = nc.dram_tensor("b1v", [128, 1], f32, kind="ExternalInput")
    d_b1b = nc.dram_tensor("b1b", [128, 1], f32, kind="ExternalInput")
    d_b2 = nc.dram_tensor("b2v", [128, 1], f32, kind="ExternalInput")
    d_b3 = nc.dram_tensor("b3v", [8, 1], f32, kind="ExternalInput")
    d_zcf = nc.dram_tensor("zcf", [128, BC], fh, kind="ExternalInput")
    d_out2 = nc.dram_tensor("out2", [T // 4, 4, 8, BC], fh, kind="ExternalOutput")
    d_warm = nc.dram_tensor("warm", [8, 16], fh, kind="ExternalOutput")

    with tile.TileContext(nc) as tc:
        with (
            tc.tile_pool(name="const", bufs=1) as cpool,
            tc.tile_pool(name="stagp", bufs=1) as spool,
            tc.tile_pool(name="upool", bufs=8) as upool,
            tc.tile_pool(name="hpool", bufs=2) as hpool,
            tc.tile_pool(name="ph1", bufs=2, space="PSUM") as ph1p,
            tc.tile_pool(name="ph2", bufs=2, space="PSUM") as ph2p,
            tc.tile_pool(name="pyp", bufs=2, space="PSUM") as pypp,
            tc.tile_pool(name="pwarm", bufs=1, space="PSUM") as pwarmp,
        ):
            t_cm = cpool.tile_from(d_cmats[:])
            t_w2 = cpool.tile_from(d_w2[:])
            t_wc = cpool.tile_from(d_wc[:])
            t_w3 = cpool.tile_from(d_w3[:])
            t_bs = cpool.tile_from(d_bs[:])
            t_zc = cpool.tile_from(d_zcf[:])
            t_b1 = cpool.tile_from(d_b1[:])
            t_b1b = cpool.tile_from(d_b1b[:])
            t_b2 = cpool.tile_from(d_b2[:])
            t_b3 = cpool.tile_from(d_b3[:])

            stag = spool.tile([128, BC], fh, name="stag")
            nc.sync.dma_start(stag[:], d_stag0[:])

            # --- PE clock warm-up: ~6us of back-to-back matmuls trips the
            # HAM clock gate from 1.2 GHz (cold K=4/8) to 2.4 GHz before the
            # recurrence starts.  Results land in a scratch PSUM tile whose
            # corner is exported so the chain stays live.
            warm_p = pwarmp.tile([128, BC], f32, name="warmp")
            for _ in range(10):
                nc.tensor.matmul(
                    warm_p[:, :], t_w2[:, :], t_zc[:, :],
                    start=True, stop=True, skip_group_check=True,
                )
            warm_s = cpool.tile([8, 16], fh, name="warms")
            nc.scalar.copy(warm_s[:, :], warm_p[0:8, 0:16])
            nc.sync.dma_start(d_warm[:], warm_s[:, :])

            def cmat(i):
                return t_cm[:, 128 * i:128 * i + 128]

            utiles = {}

            def prefetch_u(tt):
                ut = upool.tile([20, BC], fh, name="uw", tag="uw")
                nc.sync.dma_start(ut[:], d_uwin[tt])
                utiles[tt] = ut

            for tt in range(PF):
                prefetch_u(tt)

            CA = slice(0, CW)
            CB = slice(CW, BC)

            def emit_group_xu(tt, ph1_t):
                """y-history matmuls (split per column half so each half only
                waits on its half's staging write) + full-width u matmul."""
                cidx = 4 + tt if tt < 4 else tt % NSLOT
                for cs in (CA, CB):
                    nc.tensor.matmul(
                        ph1_t[:, cs],
                        cmat(cidx),
                        stag[:, cs],
                        start=True, stop=False, skip_group_check=True,
                    )
                nc.tensor.matmul(
                    ph1_t[:, :],
                    t_bs[:, :],
                    utiles.pop(tt)[:, :],
                    start=False, stop=(tt == 0), skip_group_check=True,
                )

            def flush(ty):
                """Export y_{ty} (slot ty%4) feature-major to DRAM; the host
                transposes to batch-major at the end."""
                s = ty % 4
                nc.sync.dma_start(d_out2[ty // 4, s], stag[32 * s:32 * s + 8, :])

            ph1_cur = ph1p.tile([128, BC], f32, name="h1p", tag="h1p")
            emit_group_xu(0, ph1_cur)

            for t in range(T):
                bias1 = t_b1b if t == 0 else t_b1

                # --- tanh1 (two half-width chunks: 1a, 1b) ---
                h1_t = hpool.tile([128, BC], fh, name="h1", tag="h1")
                for cs in (CA, CB):
                    nc.scalar.activation(
                        h1_t[:, cs], ph1_cur[:, cs], Tanh, bias=bias1[:, 0:1]
                    )

                # --- mm2 per half (2a can feed tanh2a while tanh1b runs) ---
                ph2_t = ph2p.tile([128, BC], f32, name="h2p", tag="h2p")
                for cs in (CA, CB):
                    nc.tensor.matmul(
                        ph2_t[:, cs],
                        t_w2[:, :],
                        h1_t[:, cs],
                    )

                # --- next step's x-side matmuls (off the critical chain;
                #     emitted before this step's staging writes so the stale
                #     y_{t-3} slot read stays dependency-free) ---
                ph1_next = None
                if t + 1 < T:
                    ph1_next = ph1p.tile([128, BC], f32, name="h1p", tag="h1p")
                    emit_group_xu(t + 1, ph1_next)

                # --- tanh2 (chunks 2a, 2b) ---
                h2_t = hpool.tile([128, BC], fh, name="h2", tag="h2")
                for cs in (CA, CB):
                    nc.scalar.activation(
                        h2_t[:, cs], ph2_t[:, cs], Tanh, bias=t_b2[:, 0:1]
                    )

                # --- output flush (1 slot/step; ~3 steps of slack) ---
                if t >= 1:
                    flush(t - 1)

                # --- close ph1_next and produce y_{t+1}, per half:
                #     mmCa, mm3a, mm3b, mmCb -- mm3b ahead of mmCb so the
                #     b-half staging write lands before mm1b(t+2) needs it ---
                if t + 1 < T:
                    pyp_t = pypp.tile([8, BC], f32, name="yp", tag="yp")
                    nc.tensor.matmul(
                        ph1_next[:, CA], t_wc[:, :], h2_t[:, CA],
                        start=False, stop=True, skip_group_check=True,
                    )
                    nc.tensor.matmul(pyp_t[:, CA], t_w3[:, :], h2_t[:, CA])
                    nc.tensor.matmul(pyp_t[:, CB], t_w3[:, :], h2_t[:, CB])
                    nc.tensor.matmul(
                        ph1_next[:, CB], t_wc[:, :], h2_t[:, CB],
                        start=False, stop=True, skip_group_check=True,
                    )
                    s_new = (t + 1) % NSLOT
                    srow = stag[32 * s_new:32 * s_new + 8, :]
                    nc.vector.tensor_scalar_add(
                        srow[:, CA], pyp_t[:, CA], t_b3[:, 0:1]
                    )
                    nc.vector.tensor_scalar_add(
                        srow[:, CB], pyp_t[:, CB], t_b3[:, 0:1]
                    )

                if t + PF < T:
                    prefetch_u(t + PF)

                ph1_cur = ph1_next

            flush(T - 1)

    nc.compile()
    return nc


def _host_prep(useq, yz0, W1, b1, W2, b2, W3, b3):
    """Build the per-core input maps (all host-side numpy)."""
    useq = np.ascontiguousarray(useq, dtype=np.float32)
    yz0 = np.ascontiguousarray(yz0, dtype=np.float32)
    W1 = np.asarray(W1, dtype=np.float32)
    W2 = np.ascontiguousarray(W2, dtype=np.float32)
    W3 = np.ascontiguousarray(W3, dtype=np.float32)
    b1 = np.asarray(b1, dtype=np.float32)
    b2 = np.asarray(b2, dtype=np.float32)
    b3 = np.asarray(b3, dtype=np.float32)

    A = {0: W1[0:8], 4: W1[8:16], 3: W1[16:24], 2: W1[24:32], 1: W1[32:40]}
    Bstack = W1[40:60]  # u_{t-4..t} stacked chronologically

    # staging rows: slot s -> [32s, 32s+8) holds y ring;
    #               boot block s -> [32s+8, 32s+16) holds y_{-(s+1)}
    cmats = np.zeros((8, 128, 128), dtype=np.float32)
    for p in range(NSLOT):  # steady phases, t >= 4: every slot one A_k
        for s in range(NSLOT):
            k = ((p - s - 1) % 4) + 1
            cmats[p, 32 * s:32 * s + 8] = A[k]
    for tt in range(4):  # boot steps t=0..3
        cb = cmats[4 + tt]
        for k in range(1, 5):
            if tt - k >= 0:
                s = (tt - k) % 4
                cb[32 * s:32 * s + 8] += A[k]
            else:
                s = k - tt - 1
                cb[32 * s + 8:32 * s + 16] += A[k]
        if tt == 0:
            cb[0:8] += A[0]  # slot 0 carries y_0 directly at t=0
    cmats2d = np.ascontiguousarray(
        cmats.transpose(1, 0, 2).reshape(128, 8 * 128)
    )

    WC = np.ascontiguousarray(W3 @ A[0])          # [128, 128]
    b1_eff = (b1 + A[0].T @ b3).reshape(128, 1)   # mmC path lacks A0^T b3
    b1_boot = b1.reshape(128, 1)
    b2v = b2.reshape(128, 1)
    b3v = b3.reshape(8, 1)
    ident = np.eye(128, dtype=np.float16)

    in_maps = []
    for c in range(NCORES):
        bs = slice(c * BC, (c + 1) * BC)
        u_c = useq[bs]      # [BC, T, 4]
        yz_c = yz0[bs]      # [BC, 56]

        stag0 = np.zeros((128, BC), dtype=np.float32)
        stag0[0:8] = yz_c[:, 0:8].T               # slot 0 = y_0
        for s in range(4):                         # boot blocks y_{-(s+1)}
            blk = yz_c[:, 8 + 8 * (3 - s):16 + 8 * (3 - s)]  # ypseq newest last
            stag0[32 * s + 8:32 * s + 16] = blk.T

        # sliding u-windows for the K=20 u matmul
        uhist = yz_c[:, 40:56].reshape(BC, 4, 4)          # u_{-4..-1}
        uext = np.concatenate([uhist, u_c], axis=1)       # [BC, T+4, 4]
        sw = np.lib.stride_tricks.sliding_window_view(uext, 5, axis=1)
        # sw: [BC, T, 4, 5] -> uwin [T, 20, BC] (chronological rows)
        uwin = np.ascontiguousarray(sw.transpose(1, 3, 2, 0).reshape(T, 20, BC))

        in_maps.append({
            "stag0": stag0.astype(np.float16),
            "uwin": uwin.astype(np.float16),
            "bs": np.ascontiguousarray(Bstack).astype(np.float16),
            "zcf": np.zeros((128, BC), dtype=np.float16),
            "cmats": cmats2d.astype(np.float16),
            "w2": W2.astype(np.float16),
            "wc": WC.astype(np.float16),
            "w3": W3.astype(np.float16),
            "b1v": np.ascontiguousarray(b1_eff),
            "b1b": np.ascontiguousarray(b1_boot),
            "b2v": np.ascontiguousarray(b2v),
            "b3v": np.ascontiguousarray(b3v),
        })
    return in_maps


def get_program():
    if "nc" not in _COMPILED:
        _COMPILED["nc"] = _build_program()
    return _COMPILED["nc"]


def _enable_ldw_opt():
    """Allow walrus to double-buffer LDWEIGHTS (background weight loads).

    The environment default is --enable-ldw-opt=false, which serializes
    every LDWEIGHTS behind the previous matmul's drain; with ~6 weight
    switches per RNN step that costs ~2x on the tensor engine.
    """
    try:
        from concourse.compiler_utils import get_compiler_flags, set_compiler_flags

        flags = get_compiler_flags()
        new = [f.replace("--enable-ldw-opt=false", "--enable-ldw-opt=true") for f in flags]
        if new != flags:
            set_compiler_flags(new)
    except Exception:
        pass


def run_cores(in_maps, **kwargs):
    from concourse.bass_utils import run_bass_kernel_spmd

    _enable_ldw_opt()
    nc = get_program()
    return run_bass_kernel_spmd(nc, in_maps, core_ids=list(range(NCORES)), **kwargs)


def assemble(res):
    outs = []
    for r in res.results:
        buf = np.asarray(r["out2"], dtype=np.float32)   # [T/4, 4, 8, BC]
        ys = buf.transpose(3, 0, 1, 2).reshape(BC, T, NY)
        outs.append(ys)
    return np.concatenate(outs, axis=0)


def kernel(useq, yz0, W1, b1, W2, b2, W3, b3):
    in_maps = _host_prep(useq, yz0, W1, b1, W2, b2, W3, b3)
    res = run_cores(in_maps)
    return assemble(res)



# revision 20
# speedup vs baseline: 1.1242x; 1.1242x over previous
"""Trainium2 Bass kernel for the NP/NY/NU RNN scan (nn_BlackBoxModel_24489903521937).

Model (per step t, batch row b):
    x_t   = [y_t, y_{t-4..t-1}, u_{t-4..t-1}, u_t]          (60)
    h1    = tanh(x_t @ W1 + b1)                              (128)
    h2    = tanh(h1 @ W2 + b2)                               (128)
    y_{t+1} = h2 @ W3 + b3                                   (8)
    output ys[:, t] = y_t

Strategy (pure data parallel, batch 4096 -> 8 cores x 512):
  * feature-major layout: features on SBUF partitions, batch on the free dim.
  * y-history lives in 4 ring slots of a [128, B] staging tile, one slot per
    32-partition strip (SBUF APs must start at partition 0/32/64/96).  The
    x @ W1 product becomes: one K=128 matmul against phase-permuted W1 blocks
    (C_p, zero rows where a slot is semantically dead), one K=20 sliding
    u-window matmul, and one composed (W3 @ A0) matmul from h2 directly, so
    the recurrent cycle is just tanh -> mm(W2) -> tanh -> mm(W3 A0).
  * y_{t-4} is read from the slot y_t is about to overwrite: emission order
    (mmX before the staging write) makes Tile sequence the write after the
    read, so no extra buffering is needed.
  * outputs retire from the staging tile by raw feature-major DMA every 4
    steps; the host does the final [T,8,B] -> [B,T,8] transpose.
  * matmul operands are fp16 (1 cycle/row, fp32 PSUM accumulate); the
    5-step fading memory of the state keeps fp16 error flat (~6e-4).
"""

import numpy as np

NP_, NY, NU = 4, 8, 4
B, T, H = 4096, 256, 128
NCORES = 8
BC = B // NCORES  # 512 batch rows per core
CHUNKS = 2        # column chunks for the critical tanh/matmul cycle
CW = BC // CHUNKS
PF = 6            # u-window DMA prefetch depth (steps ahead)
NSLOT = 4         # y ring slots (one per 32-partition strip)

_COMPILED = {}


def _build_program():
    import concourse.mybir as mybir
    import concourse.tile as tile
    from concourse import bacc

    f32 = mybir.dt.float32
    fh = mybir.dt.float16
    Tanh = mybir.ActivationFunctionType.Tanh

    nc = bacc.Bacc("TRN2", target_bir_lowering=False, debug=False)

    d_stag0 = nc.dram_tensor("stag0", [128, BC], fh, kind="ExternalInput")
    d_uwin = nc.dram_tensor("uwin", [T, 20, BC], fh, kind="ExternalInput")
    d_bs = nc.dram_tensor("bs", [20, 128], fh, kind="ExternalInput")
    # 8 C matrices: [0..3] steady phases (t % 4), [4..7] boot steps t=0..3
    d_cmats = nc.dram_tensor("cmats", [128, 8 * 128], fh, kind="ExternalInput")
    d_w2 = nc.dram_tensor("w2", [128, 128], fh, kind="ExternalInput")
    d_wc = nc.dram_tensor("wc", [128, 128], fh, kind="ExternalInput")
    d_w3 = nc.dram_tensor("w3", [128, 8], fh, kind="ExternalInput")
    d_b1 = nc.dram_tensor("b1v", [128, 1], f32, kind="ExternalInput")
    d_b1b = nc.dram_tensor("b1b", [128, 1], f32, kind="ExternalInput")
    d_b2 = nc.dram_tensor("b2v", [128, 1], f32, kind="ExternalInput")
    d_b3 = nc.dram_tensor("b3v", [8, 1], f32, kind="ExternalInput")
    d_zcf = nc.dram_tensor("zcf", [128, BC], fh, kind="ExternalInput")
    d_out2 = nc.dram_tensor("out2", [T // 4, 4, 8, BC], fh, kind="ExternalOutput")
    d_warm = nc.dram_tensor("warm", [8, 16], fh, kind="ExternalOutput")

    with tile.TileContext(nc) as tc:
        with (
            tc.tile_pool(name="const", bufs=1) as cpool,
            tc.tile_pool(name="stagp", bufs=1) as spool,
            tc.tile_pool(name="upool", bufs=8) as upool,
            tc.tile_pool(name="hpool", bufs=2) as hpool,
            tc.tile_pool(name="ph1", bufs=2, space="PSUM") as ph1p,
            tc.tile_pool(name="ph2", bufs=2, space="PSUM") as ph2p,
            tc.tile_pool(name="pyp", bufs=2, space="PSUM") as pypp,
            tc.tile_pool(name="pwarm", bufs=1, space="PSUM") as pwarmp,
        ):
            t_cm = cpool.tile_from(d_cmats[:])
            t_w2 = cpool.tile_from(d_w2[:])
            t_wc = cpool.tile_from(d_wc[:])
            t_w3 = cpool.tile_from(d_w3[:])
            t_bs = cpool.tile_from(d_bs[:])
            t_zc = cpool.tile_from(d_zcf[:])
            t_b1 = cpool.tile_from(d_b1[:])
            t_b1b = cpool.tile_from(d_b1b[:])
            t_b2 = cpool.tile_from(d_b2[:])
            t_b3 = cpool.tile_from(d_b3[:])

            stag = spool.tile([128, BC], fh, name="stag")
            nc.sync.dma_start(stag[:], d_stag0[:])

            # --- PE clock warm-up: ~6us of back-to-back matmuls trips the
            # HAM clock gate from 1.2 GHz (cold K=4/8) to 2.4 GHz before the
            # recurrence starts.  Results land in a scratch PSUM tile whose
            # corner is exported so the chain stays live.
            warm_p = pwarmp.tile([128, BC], f32, name="warmp")
            for _ in range(10):
                nc.tensor.matmul(
                    warm_p[:, :], t_w2[:, :], t_zc[:, :],
                    start=True, stop=True, skip_group_check=True,
                )
            warm_s = cpool.tile([8, 16], fh, name="warms")
            nc.scalar.copy(warm_s[:, :], warm_p[0:8, 0:16])
            nc.sync.dma_start(d_warm[:], warm_s[:, :])

            def cmat(i):
                return t_cm[:, 128 * i:128 * i + 128]

            utiles = {}

            def prefetch_u(tt):
                ut = upool.tile([20, BC], fh, name="uw", tag="uw")
                nc.sync.dma_start(ut[:], d_uwin[tt])
                utiles[tt] = ut

            for tt in range(PF):
                prefetch_u(tt)

            CA = slice(0, CW)
            CB = slice(CW, BC)

            def emit_group_xu(tt, ph1_t):
                """u-window matmul FIRST (start=True -- the one bank-wide
                has_written clear for this group), then the full-width
                y-history matmul accumulating onto it.  Exactly one start and
                one stop per PSUM bank group (start=True clears the whole
                bank's has_written bits, so per-half starts corrupt the
                group)."""
                cidx = 4 + tt if tt < 4 else tt % NSLOT
                nc.tensor.matmul(
                    ph1_t[:, :],
                    t_bs[:, :],
                    utiles.pop(tt)[:, :],
                    start=True, stop=False, skip_group_check=True,
                )
                nc.tensor.matmul(
                    ph1_t[:, :],
                    cmat(cidx),
                    stag[:, :],
                    start=False, stop=(tt == 0), skip_group_check=True,
                )

            def flush(ty):
                """Export y_{ty} (slot ty%4) feature-major to DRAM; the host
                transposes to batch-major at the end."""
                s = ty % 4
                nc.sync.dma_start(d_out2[ty // 4, s], stag[32 * s:32 * s + 8, :])

            ph1_cur = ph1p.tile([128, BC], f32, name="h1p", tag="h1p")
            emit_group_xu(0, ph1_cur)

            for t in range(T):
                bias1 = t_b1b if t == 0 else t_b1

                # --- tanh1 (two half-width chunks: 1a, 1b) ---
                h1_t = hpool.tile([128, BC], fh, name="h1", tag="h1")
                for cs in (CA, CB):
                    nc.scalar.activation(
                        h1_t[:, cs], ph1_cur[:, cs], Tanh, bias=bias1[:, 0:1]
                    )

                # --- mm2 per half (2a can feed tanh2a while tanh1b runs) ---
                ph2_t = ph2p.tile([128, BC], f32, name="h2p", tag="h2p")
                for cs in (CA, CB):
                    nc.tensor.matmul(
                        ph2_t[:, cs],
                        t_w2[:, :],
                        h1_t[:, cs],
                    )

                # --- next step's x-side matmuls (off the critical chain;
                #     emitted before this step's staging writes so the stale
                #     y_{t-3} slot read stays dependency-free) ---
                ph1_next = None
                if t + 1 < T:
                    ph1_next = ph1p.tile([128, BC], f32, name="h1p", tag="h1p")
                    emit_group_xu(t + 1, ph1_next)

                # --- tanh2 (chunks 2a, 2b) ---
                h2_t = hpool.tile([128, BC], fh, name="h2", tag="h2")
                for cs in (CA, CB):
                    nc.scalar.activation(
                        h2_t[:, cs], ph2_t[:, cs], Tanh, bias=t_b2[:, 0:1]
                    )

                # --- output flush (1 slot/step; ~3 steps of slack) ---
                if t >= 1:
                    flush(t - 1)

                # --- close ph1_next and produce y_{t+1}, per half:
                #     mmCa, mm3a, mm3b, mmCb -- mm3b ahead of mmCb so the
                #     b-half staging write lands before mm1b(t+2) needs it ---
                if t + 1 < T:
                    pyp_t = pypp.tile([8, BC], f32, name="yp", tag="yp")
                    nc.tensor.matmul(
                        ph1_next[:, CA], t_wc[:, :], h2_t[:, CA],
                        start=False, stop=False, skip_group_check=True,
                    )
                    nc.tensor.matmul(pyp_t[:, CA], t_w3[:, :], h2_t[:, CA])
                    nc.tensor.matmul(pyp_t[:, CB], t_w3[:, :], h2_t[:, CB])
                    nc.tensor.matmul(
                        ph1_next[:, CB], t_wc[:, :], h2_t[:, CB],
                        start=False, stop=True, skip_group_check=True,
                    )
                    p0 = 32 * ((t + 1) % NSLOT)
                    nc.vector.tensor_scalar_add(
                        stag[p0:p0 + 8, CA], pyp_t[:, CA], t_b3[:, 0:1]
                    )
                    nc.vector.tensor_scalar_add(
                        stag[p0:p0 + 8, CB], pyp_t[:, CB], t_b3[:, 0:1]
                    )

                if t + PF < T:
                    prefetch_u(t + PF)

                ph1_cur = ph1_next

            flush(T - 1)

    nc.compile()
    return nc


def _host_prep(useq, yz0, W1, b1, W2, b2, W3, b3):
    """Build the per-core input maps (all host-side numpy)."""
    useq = np.ascontiguousarray(useq, dtype=np.float32)
    yz0 = np.ascontiguousarray(yz0, dtype=np.float32)
    W1 = np.asarray(W1, dtype=np.float32)
    W2 = np.ascontiguousarray(W2, dtype=np.float32)
    W3 = np.ascontiguousarray(W3, dtype=np.float32)
    b1 = np.asarray(b1, dtype=np.float32)
    b2 = np.asarray(b2, dtype=np.float32)
    b3 = np.asarray(b3, dtype=np.float32)

    A = {0: W1[0:8], 4: W1[8:16], 3: W1[16:24], 2: W1[24:32], 1: W1[32:40]}
    Bstack = W1[40:60]  # u_{t-4..t} stacked chronologically

    # staging rows: slot s -> [32s, 32s+8) holds y ring;
    #               boot block s -> [32s+8, 32s+16) holds y_{-(s+1)}
    cmats = np.zeros((8, 128, 128), dtype=np.float32)
    for p in range(NSLOT):  # steady phases, t >= 4: every slot one A_k
        for s in range(NSLOT):
            k = ((p - s - 1) % 4) + 1
            cmats[p, 32 * s:32 * s + 8] = A[k]
    for tt in range(4):  # boot steps t=0..3
        cb = cmats[4 + tt]
        for k in range(1, 5):
            if tt - k >= 0:
                s = (tt - k) % 4
                cb[32 * s:32 * s + 8] += A[k]
            else:
                s = k - tt - 1
                cb[32 * s + 8:32 * s + 16] += A[k]
        if tt == 0:
            cb[0:8] += A[0]  # slot 0 carries y_0 directly at t=0
    cmats2d = np.ascontiguousarray(
        cmats.transpose(1, 0, 2).reshape(128, 8 * 128)
    )

    WC = np.ascontiguousarray(W3 @ A[0])          # [128, 128]
    b1_eff = (b1 + A[0].T @ b3).reshape(128, 1)   # mmC path lacks A0^T b3
    b1_boot = b1.reshape(128, 1)
    b2v = b2.reshape(128, 1)
    b3v = b3.reshape(8, 1)
    ident = np.eye(128, dtype=np.float16)

    in_maps = []
    for c in range(NCORES):
        bs = slice(c * BC, (c + 1) * BC)
        u_c = useq[bs]      # [BC, T, 4]
        yz_c = yz0[bs]      # [BC, 56]

        stag0 = np.zeros((128, BC), dtype=np.float32)
        stag0[0:8] = yz_c[:, 0:8].T               # slot 0 = y_0
        for s in range(4):                         # boot blocks y_{-(s+1)}
            blk = yz_c[:, 8 + 8 * (3 - s):16 + 8 * (3 - s)]  # ypseq newest last
            stag0[32 * s + 8:32 * s + 16] = blk.T

        # sliding u-windows for the K=20 u matmul
        uhist = yz_c[:, 40:56].reshape(BC, 4, 4)          # u_{-4..-1}
        uext = np.concatenate([uhist, u_c], axis=1)       # [BC, T+4, 4]
        sw = np.lib.stride_tricks.sliding_window_view(uext, 5, axis=1)
        # sw: [BC, T, 4, 5] -> uwin [T, 20, BC] (chronological rows)
        uwin = np.ascontiguousarray(sw.transpose(1, 3, 2, 0).reshape(T, 20, BC))

        in_maps.append({
            "stag0": stag0.astype(np.float16),
            "uwin": uwin.astype(np.float16),
            "bs": np.ascontiguousarray(Bstack).astype(np.float16),
            "zcf": np.zeros((128, BC), dtype=np.float16),
            "cmats": cmats2d.astype(np.float16),
            "w2": W2.astype(np.float16),
            "wc": WC.astype(np.float16),
            "w3": W3.astype(np.float16),
            "b1v": np.ascontiguousarray(b1_eff),
            "b1b": np.ascontiguousarray(b1_boot),
            "b2v": np.ascontiguousarray(b2v),
            "b3v": np.ascontiguousarray(b3v),
        })
    return in_maps


def get_program():
    if "nc" not in _COMPILED:
        _COMPILED["nc"] = _build_program()
    return _COMPILED["nc"]


def _enable_ldw_opt():
    """Allow walrus to double-buffer LDWEIGHTS (background weight loads).

    The environment default is --enable-ldw-opt=false, which serializes
    every LDWEIGHTS behind the previous matmul's drain; with ~6 weight
    switches per RNN step that costs ~2x on the tensor engine.
    """
    try:
        from concourse.compiler_utils import get_compiler_flags, set_compiler_flags

        flags = get_compiler_flags()
        new = [f.replace("--enable-ldw-opt=false", "--enable-ldw-opt=true") for f in flags]
        if new != flags:
            set_compiler_flags(new)
    except Exception:
        pass


def run_cores(in_maps, **kwargs):
    from concourse.bass_utils import run_bass_kernel_spmd

    _enable_ldw_opt()
    nc = get_program()
    return run_bass_kernel_spmd(nc, in_maps, core_ids=list(range(NCORES)), **kwargs)


def assemble(res):
    outs = []
    for r in res.results:
        buf = np.asarray(r["out2"], dtype=np.float32)   # [T/4, 4, 8, BC]
        ys = buf.transpose(3, 0, 1, 2).reshape(BC, T, NY)
        outs.append(ys)
    return np.concatenate(outs, axis=0)


def kernel(useq, yz0, W1, b1, W2, b2, W3, b3):
    in_maps = _host_prep(useq, yz0, W1, b1, W2, b2, W3, b3)
    res = run_cores(in_maps)
    return assemble(res)



# revision 21
# speedup vs baseline: 1.3133x; 1.1682x over previous
"""Trainium2 Bass kernel for the NP/NY/NU RNN scan (nn_BlackBoxModel_24489903521937).

Model (per step t, batch row b):
    x_t   = [y_t, y_{t-4..t-1}, u_{t-4..t-1}, u_t]          (60)
    h1    = tanh(x_t @ W1 + b1)                              (128)
    h2    = tanh(h1 @ W2 + b2)                               (128)
    y_{t+1} = h2 @ W3 + b3                                   (8)
    output ys[:, t] = y_t

Strategy (pure data parallel, batch 4096 -> 8 cores x 512):
  * feature-major layout: features on SBUF partitions, batch on the free dim.
  * y-history lives in 4 ring slots of a [128, B] staging tile, one slot per
    32-partition strip (SBUF APs must start at partition 0/32/64/96).  The
    x @ W1 product becomes: one K=128 matmul against phase-permuted W1 blocks
    (C_p, zero rows where a slot is semantically dead), one K=20 sliding
    u-window matmul, and one composed (W3 @ A0) matmul from h2 directly, so
    the recurrent cycle is just tanh -> mm(W2) -> tanh -> mm(W3 A0).
  * y_{t-4} is read from the slot y_t is about to overwrite: emission order
    (mmX before the staging write) makes Tile sequence the write after the
    read, so no extra buffering is needed.
  * outputs retire from the staging tile by raw feature-major DMA every 4
    steps; the host does the final [T,8,B] -> [B,T,8] transpose.
  * matmul operands are fp16 (1 cycle/row, fp32 PSUM accumulate); the
    5-step fading memory of the state keeps fp16 error flat (~6e-4).
"""

import numpy as np

NP_, NY, NU = 4, 8, 4
B, T, H = 4096, 256, 128
NCORES = 8
BC = B // NCORES  # 512 batch rows per core
CHUNKS = 2        # column chunks for the critical tanh/matmul cycle
CW = BC // CHUNKS
PF = 6            # u-window DMA prefetch depth (steps ahead)
NSLOT = 4         # y ring slots (one per 32-partition strip)

_COMPILED = {}


def _build_program():
    import concourse.mybir as mybir
    import concourse.tile as tile
    from concourse import bacc

    f32 = mybir.dt.float32
    fh = mybir.dt.float16
    Tanh = mybir.ActivationFunctionType.Tanh

    nc = bacc.Bacc("TRN2", target_bir_lowering=False, debug=False)

    d_stag0 = nc.dram_tensor("stag0", [128, BC], fh, kind="ExternalInput")
    d_uwin = nc.dram_tensor("uwin", [T, 20, BC], fh, kind="ExternalInput")
    d_bs = nc.dram_tensor("bs", [20, 128], fh, kind="ExternalInput")
    # 8 C matrices: [0..3] steady phases (t % 4), [4..7] boot steps t=0..3
    d_cmats = nc.dram_tensor("cmats", [128, 8 * 128], fh, kind="ExternalInput")
    d_w2 = nc.dram_tensor("w2", [128, 128], fh, kind="ExternalInput")
    d_wc = nc.dram_tensor("wc", [128, 128], fh, kind="ExternalInput")
    d_w3 = nc.dram_tensor("w3", [128, 8], fh, kind="ExternalInput")
    d_b1 = nc.dram_tensor("b1v", [128, 1], f32, kind="ExternalInput")
    d_b1b = nc.dram_tensor("b1b", [128, 1], f32, kind="ExternalInput")
    d_b2 = nc.dram_tensor("b2v", [128, 1], f32, kind="ExternalInput")
    d_b3 = nc.dram_tensor("b3v", [8, 1], f32, kind="ExternalInput")
    d_zcf = nc.dram_tensor("zcf", [128, BC], fh, kind="ExternalInput")
    d_out2 = nc.dram_tensor("out2", [T // 4, 4, 8, BC], fh, kind="ExternalOutput")
    d_warm = nc.dram_tensor("warm", [8, 16], fh, kind="ExternalOutput")

    with tile.TileContext(nc) as tc:
        with (
            tc.tile_pool(name="const", bufs=1) as cpool,
            tc.tile_pool(name="stagp", bufs=1) as spool,
            tc.tile_pool(name="upool", bufs=8) as upool,
            tc.tile_pool(name="hpool", bufs=2) as hpool,
            tc.tile_pool(name="ph1a", bufs=2, space="PSUM") as ph1ap,
            tc.tile_pool(name="ph1b", bufs=2, space="PSUM") as ph1bp,
            tc.tile_pool(name="ph2", bufs=2, space="PSUM") as ph2p,
            tc.tile_pool(name="pyp", bufs=2, space="PSUM") as pypp,
        ):
            t_cm = cpool.tile_from(d_cmats[:])
            t_w2 = cpool.tile_from(d_w2[:])
            t_wc = cpool.tile_from(d_wc[:])
            t_w3 = cpool.tile_from(d_w3[:])
            t_bs = cpool.tile_from(d_bs[:])
            t_zc = cpool.tile_from(d_zcf[:])
            t_b1 = cpool.tile_from(d_b1[:])
            t_b1b = cpool.tile_from(d_b1b[:])
            t_b2 = cpool.tile_from(d_b2[:])
            t_b3 = cpool.tile_from(d_b3[:])

            stag = spool.tile([128, BC], fh, name="stag")
            nc.sync.dma_start(stag[:], d_stag0[:])

            # --- PE clock warm-up: ~6us of back-to-back matmuls trips the
            # HAM clock gate from 1.2 GHz (cold K=4/8) to 2.4 GHz before the
            # recurrence starts.  Results land in a scratch PSUM tile whose
            # corner is exported so the chain stays live.
            warm_p = ph2p.tile([128, BC], f32, name="warmp", tag="h2p")
            for _ in range(10):
                nc.tensor.matmul(
                    warm_p[:, :], t_w2[:, :], t_zc[:, :],
                    start=True, stop=True, skip_group_check=True,
                )
            warm_s = cpool.tile([8, 16], fh, name="warms")
            nc.scalar.copy(warm_s[:, :], warm_p[0:8, 0:16])
            nc.sync.dma_start(d_warm[:], warm_s[:, :])

            def cmat(i):
                return t_cm[:, 128 * i:128 * i + 128]

            utiles = {}

            def prefetch_u(tt):
                ut = upool.tile([20, BC], fh, name="uw", tag="uw")
                nc.sync.dma_start(ut[:], d_uwin[tt])
                utiles[tt] = ut

            for tt in range(PF):
                prefetch_u(tt)

            CA = slice(0, CW)
            CB = slice(CW, BC)

            def emit_group_xu(tt, phA, phB):
                """Open both half groups for step tt: per half (own PSUM
                bank) mmU (start=True, the bank-wide has_written clear) then
                the y-history matmul.  Each half group is closed later by its
                own mmC, so tanh1 of one half never waits on the other."""
                cidx = 4 + tt if tt < 4 else tt % NSLOT
                ut = utiles.pop(tt)
                for ph, cs in ((phA, CA), (phB, CB)):
                    nc.tensor.matmul(
                        ph[:, :],
                        t_bs[:, :],
                        ut[:, cs],
                        start=True, stop=False, skip_group_check=True,
                    )
                for ph, cs in ((phA, CA), (phB, CB)):
                    nc.tensor.matmul(
                        ph[:, :],
                        cmat(cidx),
                        stag[:, cs],
                        start=False, stop=(tt == 0), skip_group_check=True,
                    )

            def flush(ty):
                """Export y_{ty} (slot ty%4) feature-major to DRAM; the host
                transposes to batch-major at the end."""
                s = ty % 4
                nc.sync.dma_start(d_out2[ty // 4, s], stag[32 * s:32 * s + 8, :])

            phA_cur = ph1ap.tile([128, CW], f32, name="h1pa", tag="h1pa")
            phB_cur = ph1bp.tile([128, CW], f32, name="h1pb", tag="h1pb")
            emit_group_xu(0, phA_cur, phB_cur)

            for t in range(T):
                bias1 = t_b1b if t == 0 else t_b1

                # --- tanh1 chunks 1a, 1b ---
                h1_t = hpool.tile([128, BC], fh, name="h1", tag="h1")
                nc.scalar.activation(
                    h1_t[:, CA], phA_cur[:, :], Tanh, bias=bias1[:, 0:1]
                )
                nc.scalar.activation(
                    h1_t[:, CB], phB_cur[:, :], Tanh, bias=bias1[:, 0:1]
                )

                # --- mm2 per half ---
                ph2_t = ph2p.tile([128, BC], f32, name="h2p", tag="h2p")
                for cs in (CA, CB):
                    nc.tensor.matmul(
                        ph2_t[:, cs],
                        t_w2[:, :],
                        h1_t[:, cs],
                    )

                # --- open next step's half groups (off the critical chain;
                #     emitted before this step's staging writes so the stale
                #     y_{t-3} slot read stays dependency-free) ---
                phA_next = phB_next = None
                if t + 1 < T:
                    phA_next = ph1ap.tile([128, CW], f32, name="h1pa", tag="h1pa")
                    phB_next = ph1bp.tile([128, CW], f32, name="h1pb", tag="h1pb")
                    emit_group_xu(t + 1, phA_next, phB_next)

                # --- tanh2 chunks 2a, 2b ---
                h2_t = hpool.tile([128, BC], fh, name="h2", tag="h2")
                nc.scalar.activation(
                    h2_t[:, CA], ph2_t[:, CA], Tanh, bias=t_b2[:, 0:1]
                )
                nc.scalar.activation(
                    h2_t[:, CB], ph2_t[:, CB], Tanh, bias=t_b2[:, 0:1]
                )

                # --- output flush (1 slot/step; ~3 steps of slack) ---
                if t >= 1:
                    flush(t - 1)

                # --- close the half groups + y_{t+1} = W3^T h2 + b3:
                #     PE order mmCa, mm3a, mm3b, mmCb ---
                if t + 1 < T:
                    pyp_t = pypp.tile([8, BC], f32, name="yp", tag="yp")
                    nc.tensor.matmul(
                        phA_next[:, :], t_wc[:, :], h2_t[:, CA],
                        start=False, stop=True, skip_group_check=True,
                    )
                    nc.tensor.matmul(pyp_t[:, CA], t_w3[:, :], h2_t[:, CA])
                    nc.tensor.matmul(pyp_t[:, CB], t_w3[:, :], h2_t[:, CB])
                    nc.tensor.matmul(
                        phB_next[:, :], t_wc[:, :], h2_t[:, CB],
                        start=False, stop=True, skip_group_check=True,
                    )
                    p0 = 32 * ((t + 1) % NSLOT)
                    nc.vector.tensor_scalar_add(
                        stag[p0:p0 + 8, CA], pyp_t[:, CA], t_b3[:, 0:1]
                    )
                    nc.vector.tensor_scalar_add(
                        stag[p0:p0 + 8, CB], pyp_t[:, CB], t_b3[:, 0:1]
                    )

                if t + PF < T:
                    prefetch_u(t + PF)

                phA_cur = phA_next
                phB_cur = phB_next

            flush(T - 1)

    nc.compile()
    return nc


def _host_prep(useq, yz0, W1, b1, W2, b2, W3, b3):
    """Build the per-core input maps (all host-side numpy)."""
    useq = np.ascontiguousarray(useq, dtype=np.float32)
    yz0 = np.ascontiguousarray(yz0, dtype=np.float32)
    W1 = np.asarray(W1, dtype=np.float32)
    W2 = np.ascontiguousarray(W2, dtype=np.float32)
    W3 = np.ascontiguousarray(W3, dtype=np.float32)
    b1 = np.asarray(b1, dtype=np.float32)
    b2 = np.asarray(b2, dtype=np.float32)
    b3 = np.asarray(b3, dtype=np.float32)

    A = {0: W1[0:8], 4: W1[8:16], 3: W1[16:24], 2: W1[24:32], 1: W1[32:40]}
    Bstack = W1[40:60]  # u_{t-4..t} stacked chronologically

    # staging rows: slot s -> [32s, 32s+8) holds y ring;
    #               boot block s -> [32s+8, 32s+16) holds y_{-(s+1)}
    cmats = np.zeros((8, 128, 128), dtype=np.float32)
    for p in range(NSLOT):  # steady phases, t >= 4: every slot one A_k
        for s in range(NSLOT):
            k = ((p - s - 1) % 4) + 1
            cmats[p, 32 * s:32 * s + 8] = A[k]
    for tt in range(4):  # boot steps t=0..3
        cb = cmats[4 + tt]
        for k in range(1, 5):
            if tt - k >= 0:
                s = (tt - k) % 4
                cb[32 * s:32 * s + 8] += A[k]
            else:
                s = k - tt - 1
                cb[32 * s + 8:32 * s + 16] += A[k]
        if tt == 0:
            cb[0:8] += A[0]  # slot 0 carries y_0 directly at t=0
    cmats2d = np.ascontiguousarray(
        cmats.transpose(1, 0, 2).reshape(128, 8 * 128)
    )

    WC = np.ascontiguousarray(W3 @ A[0])          # [128, 128]
    b1_eff = (b1 + A[0].T @ b3).reshape(128, 1)   # mmC path lacks A0^T b3
    b1_boot = b1.reshape(128, 1)
    b2v = b2.reshape(128, 1)
    b3v = b3.reshape(8, 1)
    ident = np.eye(128, dtype=np.float16)

    in_maps = []
    for c in range(NCORES):
        bs = slice(c * BC, (c + 1) * BC)
        u_c = useq[bs]      # [BC, T, 4]
        yz_c = yz0[bs]      # [BC, 56]

        stag0 = np.zeros((128, BC), dtype=np.float32)
        stag0[0:8] = yz_c[:, 0:8].T               # slot 0 = y_0
        for s in range(4):                         # boot blocks y_{-(s+1)}
            blk = yz_c[:, 8 + 8 * (3 - s):16 + 8 * (3 - s)]  # ypseq newest last
            stag0[32 * s + 8:32 * s + 16] = blk.T

        # sliding u-windows for the K=20 u matmul
        uhist = yz_c[:, 40:56].reshape(BC, 4, 4)          # u_{-4..-1}
        uext = np.concatenate([uhist, u_c], axis=1)       # [BC, T+4, 4]
        sw = np.lib.stride_tricks.sliding_window_view(uext, 5, axis=1)
        # sw: [BC, T, 4, 5] -> uwin [T, 20, BC] (chronological rows)
        uwin = np.ascontiguousarray(sw.transpose(1, 3, 2, 0).reshape(T, 20, BC))

        in_maps.append({
            "stag0": stag0.astype(np.float16),
            "uwin": uwin.astype(np.float16),
            "bs": np.ascontiguousarray(Bstack).astype(np.float16),
            "zcf": np.zeros((128, BC), dtype=np.float16),
            "cmats": cmats2d.astype(np.float16),
            "w2": W2.astype(np.float16),
            "wc": WC.astype(np.float16),
            "w3": W3.astype(np.float16),
            "b1v": np.ascontiguousarray(b1_eff),
            "b1b": np.ascontiguousarray(b1_boot),
            "b2v": np.ascontiguousarray(b2v),
            "b3v": np.ascontiguousarray(b3v),
        })
    return in_maps


def get_program():
    if "nc" not in _COMPILED:
        _COMPILED["nc"] = _build_program()
    return _COMPILED["nc"]


def _enable_ldw_opt():
    """Allow walrus to double-buffer LDWEIGHTS (background weight loads).

    The environment default is --enable-ldw-opt=false, which serializes
    every LDWEIGHTS behind the previous matmul's drain; with ~6 weight
    switches per RNN step that costs ~2x on the tensor engine.
    """
    try:
        from concourse.compiler_utils import get_compiler_flags, set_compiler_flags

        flags = get_compiler_flags()
        new = [f.replace("--enable-ldw-opt=false", "--enable-ldw-opt=true") for f in flags]
        if new != flags:
            set_compiler_flags(new)
    except Exception:
        pass


def run_cores(in_maps, **kwargs):
    from concourse.bass_utils import run_bass_kernel_spmd

    _enable_ldw_opt()
    nc = get_program()
    return run_bass_kernel_spmd(nc, in_maps, core_ids=list(range(NCORES)), **kwargs)


def assemble(res):
    outs = []
    for r in res.results:
        buf = np.asarray(r["out2"], dtype=np.float32)   # [T/4, 4, 8, BC]
        ys = buf.transpose(3, 0, 1, 2).reshape(BC, T, NY)
        outs.append(ys)
    return np.concatenate(outs, axis=0)


def kernel(useq, yz0, W1, b1, W2, b2, W3, b3):
    in_maps = _host_prep(useq, yz0, W1, b1, W2, b2, W3, b3)
    res = run_cores(in_maps)
    return assemble(res)



# revision 22
# speedup vs baseline: 1.5493x; 1.1797x over previous
"""Trainium2 Bass kernel for the NP/NY/NU RNN scan (nn_BlackBoxModel_24489903521937).

Model (per step t, batch row b):
    x_t   = [y_t, y_{t-4..t-1}, u_{t-4..t-1}, u_t]          (60)
    h1    = tanh(x_t @ W1 + b1)                              (128)
    h2    = tanh(h1 @ W2 + b2)                               (128)
    y_{t+1} = h2 @ W3 + b3                                   (8)
    output ys[:, t] = y_t

Strategy (pure data parallel, batch 4096 -> 8 cores x 512):
  * feature-major layout: features on SBUF partitions, batch on the free dim.
  * y-history lives in 4 ring slots of a [128, B] staging tile, one slot per
    32-partition strip (SBUF APs must start at partition 0/32/64/96).  The
    x @ W1 product becomes: one K=128 matmul against phase-permuted W1 blocks
    (C_p, zero rows where a slot is semantically dead), one K=20 sliding
    u-window matmul, and one composed (W3 @ A0) matmul from h2 directly, so
    the recurrent cycle is just tanh -> mm(W2) -> tanh -> mm(W3 A0).
  * y_{t-4} is read from the slot y_t is about to overwrite: emission order
    (mmX before the staging write) makes Tile sequence the write after the
    read, so no extra buffering is needed.
  * outputs retire from the staging tile by raw feature-major DMA every 4
    steps; the host does the final [T,8,B] -> [B,T,8] transpose.
  * matmul operands are fp16 (1 cycle/row, fp32 PSUM accumulate); the
    5-step fading memory of the state keeps fp16 error flat (~6e-4).
"""

import numpy as np

NP_, NY, NU = 4, 8, 4
B, T, H = 4096, 256, 128
NCORES = 8
BC = B // NCORES  # 512 batch rows per core
CHUNKS = 2        # column chunks for the critical tanh/matmul cycle
CW = BC // CHUNKS
PF = 6            # u-window DMA prefetch depth (steps ahead)
NSLOT = 4         # y ring slots (one per 32-partition strip)

_COMPILED = {}


def _build_program():
    import concourse.mybir as mybir
    import concourse.tile as tile
    from concourse import bacc

    f32 = mybir.dt.float32
    fh = mybir.dt.float16
    Tanh = mybir.ActivationFunctionType.Tanh

    nc = bacc.Bacc("TRN2", target_bir_lowering=False, debug=False)

    d_stag0 = nc.dram_tensor("stag0", [128, BC], fh, kind="ExternalInput")
    d_uwin = nc.dram_tensor("uwin", [T, 20, BC], fh, kind="ExternalInput")
    d_bs = nc.dram_tensor("bs", [20, 128], fh, kind="ExternalInput")
    # 8 C matrices: [0..3] steady phases (t % 4), [4..7] boot steps t=0..3
    d_cmats = nc.dram_tensor("cmats", [128, 8 * 128], fh, kind="ExternalInput")
    d_w2 = nc.dram_tensor("w2", [128, 128], fh, kind="ExternalInput")
    d_wc = nc.dram_tensor("wc", [128, 128], fh, kind="ExternalInput")
    d_w3 = nc.dram_tensor("w3", [128, 8], fh, kind="ExternalInput")
    d_b1 = nc.dram_tensor("b1v", [128, 1], f32, kind="ExternalInput")
    d_b1b = nc.dram_tensor("b1b", [128, 1], f32, kind="ExternalInput")
    d_b2 = nc.dram_tensor("b2v", [128, 1], f32, kind="ExternalInput")
    d_b3 = nc.dram_tensor("b3v", [8, 1], f32, kind="ExternalInput")
    d_zcf = nc.dram_tensor("zcf", [128, BC], fh, kind="ExternalInput")
    d_out2 = nc.dram_tensor("out2", [T // 4, 4, 8, BC], fh, kind="ExternalOutput")
    d_warm = nc.dram_tensor("warm", [8, 16], fh, kind="ExternalOutput")

    with tile.TileContext(nc) as tc:
        with (
            tc.tile_pool(name="const", bufs=1) as cpool,
            tc.tile_pool(name="stagp", bufs=1) as spool,
            tc.tile_pool(name="upool", bufs=8) as upool,
            tc.tile_pool(name="hpool", bufs=2) as hpool,
            tc.tile_pool(name="ph1a", bufs=2, space="PSUM") as ph1ap,
            tc.tile_pool(name="ph1b", bufs=2, space="PSUM") as ph1bp,
            tc.tile_pool(name="ph2a", bufs=1, space="PSUM") as ph2ap,
            tc.tile_pool(name="ph2b", bufs=1, space="PSUM") as ph2bp,
            tc.tile_pool(name="pypa", bufs=1, space="PSUM") as pypap,
            tc.tile_pool(name="pypb", bufs=1, space="PSUM") as pypbp,
        ):
            t_cm = cpool.tile_from(d_cmats[:])
            t_w2 = cpool.tile_from(d_w2[:])
            t_wc = cpool.tile_from(d_wc[:])
            t_w3 = cpool.tile_from(d_w3[:])
            t_bs = cpool.tile_from(d_bs[:])
            t_zc = cpool.tile_from(d_zcf[:])
            t_b1 = cpool.tile_from(d_b1[:])
            t_b1b = cpool.tile_from(d_b1b[:])
            t_b2 = cpool.tile_from(d_b2[:])
            t_b3 = cpool.tile_from(d_b3[:])

            stag = spool.tile([128, BC], fh, name="stag")
            nc.sync.dma_start(stag[:], d_stag0[:])

            # --- PE clock warm-up: ~6us of back-to-back matmuls trips the
            # HAM clock gate from 1.2 GHz (cold K=4/8) to 2.4 GHz before the
            # recurrence starts.  Results land in a scratch PSUM tile whose
            # corner is exported so the chain stays live.
            warm_p = ph2ap.tile([128, BC], f32, name="warmp", tag="h2pa")
            for _ in range(10):
                nc.tensor.matmul(
                    warm_p[:, :], t_w2[:, :], t_zc[:, :],
                    start=True, stop=True, skip_group_check=True,
                )
            warm_s = cpool.tile([8, 16], fh, name="warms")
            nc.scalar.copy(warm_s[:, :], warm_p[0:8, 0:16])
            nc.sync.dma_start(d_warm[:], warm_s[:, :])

            def cmat(i):
                return t_cm[:, 128 * i:128 * i + 128]

            utiles = {}

            def prefetch_u(tt):
                ut = upool.tile([20, BC], fh, name="uw", tag="uw")
                nc.sync.dma_start(ut[:], d_uwin[tt])
                utiles[tt] = ut

            for tt in range(PF):
                prefetch_u(tt)

            CA = slice(0, CW)
            CB = slice(CW, BC)

            def emit_group_xu(tt, phA, phB):
                """Open both half groups for step tt: per half (own PSUM
                bank) mmU (start=True, the bank-wide has_written clear) then
                the y-history matmul.  Each half group is closed later by its
                own mmC, so tanh1 of one half never waits on the other."""
                cidx = 4 + tt if tt < 4 else tt % NSLOT
                ut = utiles.pop(tt)
                for ph, cs in ((phA, CA), (phB, CB)):
                    nc.tensor.matmul(
                        ph[:, :],
                        t_bs[:, :],
                        ut[:, cs],
                        start=True, stop=False, skip_group_check=True,
                    )
                for ph, cs in ((phA, CA), (phB, CB)):
                    nc.tensor.matmul(
                        ph[:, :],
                        cmat(cidx),
                        stag[:, cs],
                        start=False, stop=(tt == 0), skip_group_check=True,
                    )

            def flush(ty):
                """Export y_{ty} (slot ty%4) feature-major to DRAM; the host
                transposes to batch-major at the end."""
                s = ty % 4
                nc.sync.dma_start(d_out2[ty // 4, s], stag[32 * s:32 * s + 8, :])

            phA_cur = ph1ap.tile([128, CW], f32, name="h1pa", tag="h1pa")
            phB_cur = ph1bp.tile([128, CW], f32, name="h1pb", tag="h1pb")
            emit_group_xu(0, phA_cur, phB_cur)

            for t in range(T):
                bias1 = t_b1b if t == 0 else t_b1

                # --- tanh1 chunks 1a, 1b ---
                h1_t = hpool.tile([128, BC], fh, name="h1", tag="h1")
                nc.scalar.activation(
                    h1_t[:, CA], phA_cur[:, :], Tanh, bias=bias1[:, 0:1]
                )
                nc.scalar.activation(
                    h1_t[:, CB], phB_cur[:, :], Tanh, bias=bias1[:, 0:1]
                )

                # --- mm2 per half (separate PSUM banks so tanh2a only
                #     depends on mm2a) ---
                ph2a_t = ph2ap.tile([128, CW], f32, name="h2pa", tag="h2pa")
                ph2b_t = ph2bp.tile([128, CW], f32, name="h2pb", tag="h2pb")
                nc.tensor.matmul(ph2a_t[:, :], t_w2[:, :], h1_t[:, CA])
                nc.tensor.matmul(ph2b_t[:, :], t_w2[:, :], h1_t[:, CB])

                # --- open next step's half groups (off the critical chain;
                #     emitted before this step's staging writes so the stale
                #     y_{t-3} slot read stays dependency-free) ---
                phA_next = phB_next = None
                if t + 1 < T:
                    phA_next = ph1ap.tile([128, CW], f32, name="h1pa", tag="h1pa")
                    phB_next = ph1bp.tile([128, CW], f32, name="h1pb", tag="h1pb")
                    emit_group_xu(t + 1, phA_next, phB_next)

                # --- tanh2 chunks 2a, 2b ---
                h2_t = hpool.tile([128, BC], fh, name="h2", tag="h2")
                nc.scalar.activation(
                    h2_t[:, CA], ph2a_t[:, :], Tanh, bias=t_b2[:, 0:1]
                )
                nc.scalar.activation(
                    h2_t[:, CB], ph2b_t[:, :], Tanh, bias=t_b2[:, 0:1]
                )

                # --- output flush (1 slot/step; ~3 steps of slack) ---
                if t >= 1:
                    flush(t - 1)

                # --- close the half groups + y_{t+1} = W3^T h2 + b3:
                #     PE order mmCa, mm3a, mm3b, mmCb ---
                if t + 1 < T:
                    pypa_t = pypap.tile([8, CW], f32, name="ypa", tag="ypa")
                    pypb_t = pypbp.tile([8, CW], f32, name="ypb", tag="ypb")
                    nc.tensor.matmul(
                        phA_next[:, :], t_wc[:, :], h2_t[:, CA],
                        start=False, stop=True, skip_group_check=True,
                    )
                    nc.tensor.matmul(pypa_t[:, :], t_w3[:, :], h2_t[:, CA])
                    nc.tensor.matmul(
                        phB_next[:, :], t_wc[:, :], h2_t[:, CB],
                        start=False, stop=True, skip_group_check=True,
                    )
                    nc.tensor.matmul(pypb_t[:, :], t_w3[:, :], h2_t[:, CB])
                    p0 = 32 * ((t + 1) % NSLOT)
                    nc.vector.tensor_scalar_add(
                        stag[p0:p0 + 8, CA], pypa_t[:, :], t_b3[:, 0:1]
                    )
                    nc.vector.tensor_scalar_add(
                        stag[p0:p0 + 8, CB], pypb_t[:, :], t_b3[:, 0:1]
                    )

                if t + PF < T:
                    prefetch_u(t + PF)

                phA_cur = phA_next
                phB_cur = phB_next

            flush(T - 1)

    nc.compile()
    return nc


def _host_prep(useq, yz0, W1, b1, W2, b2, W3, b3):
    """Build the per-core input maps (all host-side numpy)."""
    useq = np.ascontiguousarray(useq, dtype=np.float32)
    yz0 = np.ascontiguousarray(yz0, dtype=np.float32)
    W1 = np.asarray(W1, dtype=np.float32)
    W2 = np.ascontiguousarray(W2, dtype=np.float32)
    W3 = np.ascontiguousarray(W3, dtype=np.float32)
    b1 = np.asarray(b1, dtype=np.float32)
    b2 = np.asarray(b2, dtype=np.float32)
    b3 = np.asarray(b3, dtype=np.float32)

    A = {0: W1[0:8], 4: W1[8:16], 3: W1[16:24], 2: W1[24:32], 1: W1[32:40]}
    Bstack = W1[40:60]  # u_{t-4..t} stacked chronologically

    # staging rows: slot s -> [32s, 32s+8) holds y ring;
    #               boot block s -> [32s+8, 32s+16) holds y_{-(s+1)}
    cmats = np.zeros((8, 128, 128), dtype=np.float32)
    for p in range(NSLOT):  # steady phases, t >= 4: every slot one A_k
        for s in range(NSLOT):
            k = ((p - s - 1) % 4) + 1
            cmats[p, 32 * s:32 * s + 8] = A[k]
    for tt in range(4):  # boot steps t=0..3
        cb = cmats[4 + tt]
        for k in range(1, 5):
            if tt - k >= 0:
                s = (tt - k) % 4
                cb[32 * s:32 * s + 8] += A[k]
            else:
                s = k - tt - 1
                cb[32 * s + 8:32 * s + 16] += A[k]
        if tt == 0:
            cb[0:8] += A[0]  # slot 0 carries y_0 directly at t=0
    cmats2d = np.ascontiguousarray(
        cmats.transpose(1, 0, 2).reshape(128, 8 * 128)
    )

    WC = np.ascontiguousarray(W3 @ A[0])          # [128, 128]
    b1_eff = (b1 + A[0].T @ b3).reshape(128, 1)   # mmC path lacks A0^T b3
    b1_boot = b1.reshape(128, 1)
    b2v = b2.reshape(128, 1)
    b3v = b3.reshape(8, 1)
    ident = np.eye(128, dtype=np.float16)

    in_maps = []
    for c in range(NCORES):
        bs = slice(c * BC, (c + 1) * BC)
        u_c = useq[bs]      # [BC, T, 4]
        yz_c = yz0[bs]      # [BC, 56]

        stag0 = np.zeros((128, BC), dtype=np.float32)
        stag0[0:8] = yz_c[:, 0:8].T               # slot 0 = y_0
        for s in range(4):                         # boot blocks y_{-(s+1)}
            blk = yz_c[:, 8 + 8 * (3 - s):16 + 8 * (3 - s)]  # ypseq newest last
            stag0[32 * s + 8:32 * s + 16] = blk.T

        # sliding u-windows for the K=20 u matmul
        uhist = yz_c[:, 40:56].reshape(BC, 4, 4)          # u_{-4..-1}
        uext = np.concatenate([uhist, u_c], axis=1)       # [BC, T+4, 4]
        sw = np.lib.stride_tricks.sliding_window_view(uext, 5, axis=1)
        # sw: [BC, T, 4, 5] -> uwin [T, 20, BC] (chronological rows)
        uwin = np.ascontiguousarray(sw.transpose(1, 3, 2, 0).reshape(T, 20, BC))

        in_maps.append({
            "stag0": stag0.astype(np.float16),
            "uwin": uwin.astype(np.float16),
            "bs": np.ascontiguousarray(Bstack).astype(np.float16),
            "zcf": np.zeros((128, BC), dtype=np.float16),
            "cmats": cmats2d.astype(np.float16),
            "w2": W2.astype(np.float16),
            "wc": WC.astype(np.float16),
            "w3": W3.astype(np.float16),
            "b1v": np.ascontiguousarray(b1_eff),
            "b1b": np.ascontiguousarray(b1_boot),
            "b2v": np.ascontiguousarray(b2v),
            "b3v": np.ascontiguousarray(b3v),
        })
    return in_maps


def get_program():
    if "nc" not in _COMPILED:
        _COMPILED["nc"] = _build_program()
    return _COMPILED["nc"]


def _enable_ldw_opt():
    """Allow walrus to double-buffer LDWEIGHTS (background weight loads).

    The environment default is --enable-ldw-opt=false, which serializes
    every LDWEIGHTS behind the previous matmul's drain; with ~6 weight
    switches per RNN step that costs ~2x on the tensor engine.
    """
    try:
        from concourse.compiler_utils import get_compiler_flags, set_compiler_flags

        flags = get_compiler_flags()
        new = [f.replace("--enable-ldw-opt=false", "--enable-ldw-opt=true") for f in flags]
        if new != flags:
            set_compiler_flags(new)
    except Exception:
        pass


def run_cores(in_maps, **kwargs):
    from concourse.bass_utils import run_bass_kernel_spmd

    _enable_ldw_opt()
    nc = get_program()
    return run_bass_kernel_spmd(nc, in_maps, core_ids=list(range(NCORES)), **kwargs)


def assemble(res):
    outs = []
    for r in res.results:
        buf = np.asarray(r["out2"], dtype=np.float32)   # [T/4, 4, 8, BC]
        ys = buf.transpose(3, 0, 1, 2).reshape(BC, T, NY)
        outs.append(ys)
    return np.concatenate(outs, axis=0)


def kernel(useq, yz0, W1, b1, W2, b2, W3, b3):
    in_maps = _host_prep(useq, yz0, W1, b1, W2, b2, W3, b3)
    res = run_cores(in_maps)
    return assemble(res)



# revision 23
# speedup vs baseline: 1.8551x; 1.1973x over previous
"""Trainium2 Bass kernel for the NP/NY/NU RNN scan (nn_BlackBoxModel_24489903521937).

Model (per step t, batch row b):
    x_t   = [y_t, y_{t-4..t-1}, u_{t-4..t-1}, u_t]          (60)
    h1    = tanh(x_t @ W1 + b1)                              (128)
    h2    = tanh(h1 @ W2 + b2)                               (128)
    y_{t+1} = h2 @ W3 + b3                                   (8)
    output ys[:, t] = y_t

Strategy (pure data parallel, batch 4096 -> 8 cores x 512):
  * feature-major layout: features on SBUF partitions, batch on the free dim.
  * y-history lives in 4 ring slots of a [128, B] staging tile, one slot per
    32-partition strip (SBUF APs must start at partition 0/32/64/96).  The
    x @ W1 product becomes: one K=128 matmul against phase-permuted W1 blocks
    (C_p, zero rows where a slot is semantically dead), one K=20 sliding
    u-window matmul, and one composed (W3 @ A0) matmul from h2 directly, so
    the recurrent cycle is just tanh -> mm(W2) -> tanh -> mm(W3 A0).
  * y_{t-4} is read from the slot y_t is about to overwrite: emission order
    (mmX before the staging write) makes Tile sequence the write after the
    read, so no extra buffering is needed.
  * outputs retire from the staging tile by raw feature-major DMA every 4
    steps; the host does the final [T,8,B] -> [B,T,8] transpose.
  * matmul operands are fp16 (1 cycle/row, fp32 PSUM accumulate); the
    5-step fading memory of the state keeps fp16 error flat (~6e-4).
"""

import numpy as np

NP_, NY, NU = 4, 8, 4
B, T, H = 4096, 256, 128
NCORES = 8
BC = B // NCORES  # 512 batch rows per core
CHUNKS = 2        # column chunks for the critical tanh/matmul cycle
CW = BC // CHUNKS
PF = 6            # u-window DMA prefetch depth (steps ahead)
NSLOT = 4         # y ring slots (one per 32-partition strip)

_COMPILED = {}


def _build_program(zero_bias):
    import concourse.mybir as mybir
    import concourse.tile as tile
    from concourse import bacc

    f32 = mybir.dt.float32
    fh = mybir.dt.float16
    Tanh = mybir.ActivationFunctionType.Tanh

    nc = bacc.Bacc("TRN2", target_bir_lowering=False, debug=False)

    d_stag0 = nc.dram_tensor("stag0", [128, BC], fh, kind="ExternalInput")
    d_uwin = nc.dram_tensor("uwin", [T, 20, BC], fh, kind="ExternalInput")
    d_bs = nc.dram_tensor("bs", [20, 128], fh, kind="ExternalInput")
    # 8 C matrices: [0..3] steady phases (t % 4), [4..7] boot steps t=0..3
    d_cmats = nc.dram_tensor("cmats", [128, 8 * 128], fh, kind="ExternalInput")
    d_w2 = nc.dram_tensor("w2", [128, 128], fh, kind="ExternalInput")
    d_wc = nc.dram_tensor("wc", [128, 128], fh, kind="ExternalInput")
    d_w3 = nc.dram_tensor("w3", [128, 8], fh, kind="ExternalInput")
    d_b1 = nc.dram_tensor("b1v", [128, 1], f32, kind="ExternalInput")
    d_b1b = nc.dram_tensor("b1b", [128, 1], f32, kind="ExternalInput")
    d_b2 = nc.dram_tensor("b2v", [128, 1], f32, kind="ExternalInput")
    d_b3 = nc.dram_tensor("b3v", [8, 1], f32, kind="ExternalInput")
    d_zcf = nc.dram_tensor("zcf", [128, BC], fh, kind="ExternalInput")
    d_out2 = nc.dram_tensor("out2", [T // 4, 4, 8, BC], fh, kind="ExternalOutput")
    d_warm = nc.dram_tensor("warm", [8, 16], fh, kind="ExternalOutput")

    with tile.TileContext(nc) as tc:
        with (
            tc.tile_pool(name="const", bufs=1) as cpool,
            tc.tile_pool(name="stagp", bufs=1) as spool,
            tc.tile_pool(name="upool", bufs=8) as upool,
            tc.tile_pool(name="hpool", bufs=2) as hpool,
            tc.tile_pool(name="ph1a", bufs=2, space="PSUM") as ph1ap,
            tc.tile_pool(name="ph1b", bufs=2, space="PSUM") as ph1bp,
            tc.tile_pool(name="ph2a", bufs=1, space="PSUM") as ph2ap,
            tc.tile_pool(name="ph2b", bufs=1, space="PSUM") as ph2bp,
            tc.tile_pool(name="pypa", bufs=1, space="PSUM") as pypap,
            tc.tile_pool(name="pypb", bufs=1, space="PSUM") as pypbp,
        ):
            t_cm = cpool.tile_from(d_cmats[:])
            t_w2 = cpool.tile_from(d_w2[:])
            t_wc = cpool.tile_from(d_wc[:])
            t_w3 = cpool.tile_from(d_w3[:])
            t_bs = cpool.tile_from(d_bs[:])
            t_zc = cpool.tile_from(d_zcf[:])
            t_b1 = cpool.tile_from(d_b1[:])
            t_b1b = cpool.tile_from(d_b1b[:])
            t_b2 = cpool.tile_from(d_b2[:])
            t_b3 = cpool.tile_from(d_b3[:])

            stag = spool.tile([128, BC], fh, name="stag")
            nc.sync.dma_start(stag[:], d_stag0[:])

            # --- PE clock warm-up: ~6us of back-to-back matmuls trips the
            # HAM clock gate from 1.2 GHz (cold K=4/8) to 2.4 GHz before the
            # recurrence starts.  Results land in a scratch PSUM tile whose
            # corner is exported so the chain stays live.
            warm_p = ph2ap.tile([128, BC], f32, name="warmp", tag="h2pa")
            for _ in range(10):
                nc.tensor.matmul(
                    warm_p[:, :], t_w2[:, :], t_zc[:, :],
                    start=True, stop=True, skip_group_check=True,
                )
            warm_s = cpool.tile([8, 16], fh, name="warms")
            nc.scalar.copy(warm_s[:, :], warm_p[0:8, 0:16])
            nc.sync.dma_start(d_warm[:], warm_s[:, :])

            def cmat(i):
                return t_cm[:, 128 * i:128 * i + 128]

            utiles = {}

            def prefetch_u(tt):
                ut = upool.tile([20, BC], fh, name="uw", tag="uw")
                nc.sync.dma_start(ut[:], d_uwin[tt])
                utiles[tt] = ut

            for tt in range(PF):
                prefetch_u(tt)

            CA = slice(0, CW)
            CB = slice(CW, BC)

            def emit_group_xu(tt, phA, phB):
                """Open both half groups for step tt: per half (own PSUM
                bank) mmU (start=True, the bank-wide has_written clear) then
                the y-history matmul.  Each half group is closed later by its
                own mmC, so tanh1 of one half never waits on the other."""
                cidx = 4 + tt if tt < 4 else tt % NSLOT
                ut = utiles.pop(tt)
                for ph, cs in ((phA, CA), (phB, CB)):
                    nc.tensor.matmul(
                        ph[:, :],
                        t_bs[:, :],
                        ut[:, cs],
                        start=True, stop=False, skip_group_check=True,
                    )
                for ph, cs in ((phA, CA), (phB, CB)):
                    nc.tensor.matmul(
                        ph[:, :],
                        cmat(cidx),
                        stag[:, cs],
                        start=False, stop=(tt == 0), skip_group_check=True,
                    )

            def flush(ty):
                """Export y_{ty} (slot ty%4) feature-major to DRAM; the host
                transposes to batch-major at the end."""
                s = ty % 4
                nc.sync.dma_start(d_out2[ty // 4, s], stag[32 * s:32 * s + 8, :])

            phA_cur = ph1ap.tile([128, CW], f32, name="h1pa", tag="h1pa")
            phB_cur = ph1bp.tile([128, CW], f32, name="h1pb", tag="h1pb")
            emit_group_xu(0, phA_cur, phB_cur)

            for t in range(T):
                if zero_bias:
                    bias1 = bias2 = 0.0
                else:
                    b1t = t_b1b if t == 0 else t_b1
                    bias1 = b1t[:, 0:1]
                    bias2 = t_b2[:, 0:1]

                # --- tanh1 chunks 1a, 1b ---
                h1_t = hpool.tile([128, BC], fh, name="h1", tag="h1")
                nc.scalar.activation(
                    h1_t[:, CA], phA_cur[:, :], Tanh, bias=bias1
                )
                nc.scalar.activation(
                    h1_t[:, CB], phB_cur[:, :], Tanh, bias=bias1
                )

                # --- mm2 per half (separate PSUM banks so tanh2a only
                #     depends on mm2a) ---
                ph2a_t = ph2ap.tile([128, CW], f32, name="h2pa", tag="h2pa")
                ph2b_t = ph2bp.tile([128, CW], f32, name="h2pb", tag="h2pb")
                nc.tensor.matmul(ph2a_t[:, :], t_w2[:, :], h1_t[:, CA])
                nc.tensor.matmul(ph2b_t[:, :], t_w2[:, :], h1_t[:, CB])

                # --- open next step's half groups (off the critical chain;
                #     emitted before this step's staging writes so the stale
                #     y_{t-3} slot read stays dependency-free) ---
                phA_next = phB_next = None
                if t + 1 < T:
                    phA_next = ph1ap.tile([128, CW], f32, name="h1pa", tag="h1pa")
                    phB_next = ph1bp.tile([128, CW], f32, name="h1pb", tag="h1pb")
                    emit_group_xu(t + 1, phA_next, phB_next)

                # --- tanh2 chunks 2a, 2b ---
                h2_t = hpool.tile([128, BC], fh, name="h2", tag="h2")
                nc.scalar.activation(
                    h2_t[:, CA], ph2a_t[:, :], Tanh, bias=bias2
                )
                nc.scalar.activation(
                    h2_t[:, CB], ph2b_t[:, :], Tanh, bias=bias2
                )

                # --- output flush (1 slot/step; ~3 steps of slack) ---
                if t >= 1:
                    flush(t - 1)

                # --- close the half groups + y_{t+1} = W3^T h2 + b3:
                #     PE order mmCa, mm3a, mm3b, mmCb ---
                if t + 1 < T:
                    pypa_t = pypap.tile([8, CW], f32, name="ypa", tag="ypa")
                    pypb_t = pypbp.tile([8, CW], f32, name="ypb", tag="ypb")
                    nc.tensor.matmul(
                        phA_next[:, :], t_wc[:, :], h2_t[:, CA],
                        start=False, stop=True, skip_group_check=True,
                    )
                    nc.tensor.matmul(pypa_t[:, :], t_w3[:, :], h2_t[:, CA])
                    nc.tensor.matmul(
                        phB_next[:, :], t_wc[:, :], h2_t[:, CB],
                        start=False, stop=True, skip_group_check=True,
                    )
                    nc.tensor.matmul(pypb_t[:, :], t_w3[:, :], h2_t[:, CB])
                    p0 = 32 * ((t + 1) % NSLOT)
                    nc.vector.tensor_scalar_add(
                        stag[p0:p0 + 8, CA], pypa_t[:, :], t_b3[:, 0:1]
                    )
                    nc.vector.tensor_scalar_add(
                        stag[p0:p0 + 8, CB], pypb_t[:, :], t_b3[:, 0:1]
                    )

                if t + PF < T:
                    prefetch_u(t + PF)

                phA_cur = phA_next
                phB_cur = phB_next

            flush(T - 1)

    nc.compile()
    return nc


def _host_prep(useq, yz0, W1, b1, W2, b2, W3, b3):
    """Build the per-core input maps (all host-side numpy)."""
    useq = np.ascontiguousarray(useq, dtype=np.float32)
    yz0 = np.ascontiguousarray(yz0, dtype=np.float32)
    W1 = np.asarray(W1, dtype=np.float32)
    W2 = np.ascontiguousarray(W2, dtype=np.float32)
    W3 = np.ascontiguousarray(W3, dtype=np.float32)
    b1 = np.asarray(b1, dtype=np.float32)
    b2 = np.asarray(b2, dtype=np.float32)
    b3 = np.asarray(b3, dtype=np.float32)

    A = {0: W1[0:8], 4: W1[8:16], 3: W1[16:24], 2: W1[24:32], 1: W1[32:40]}
    Bstack = W1[40:60]  # u_{t-4..t} stacked chronologically

    # staging rows: slot s -> [32s, 32s+8) holds y ring;
    #               boot block s -> [32s+8, 32s+16) holds y_{-(s+1)}
    cmats = np.zeros((8, 128, 128), dtype=np.float32)
    for p in range(NSLOT):  # steady phases, t >= 4: every slot one A_k
        for s in range(NSLOT):
            k = ((p - s - 1) % 4) + 1
            cmats[p, 32 * s:32 * s + 8] = A[k]
    for tt in range(4):  # boot steps t=0..3
        cb = cmats[4 + tt]
        for k in range(1, 5):
            if tt - k >= 0:
                s = (tt - k) % 4
                cb[32 * s:32 * s + 8] += A[k]
            else:
                s = k - tt - 1
                cb[32 * s + 8:32 * s + 16] += A[k]
        if tt == 0:
            cb[0:8] += A[0]  # slot 0 carries y_0 directly at t=0
    cmats2d = np.ascontiguousarray(
        cmats.transpose(1, 0, 2).reshape(128, 8 * 128)
    )

    WC = np.ascontiguousarray(W3 @ A[0])          # [128, 128]
    b1_eff = (b1 + A[0].T @ b3).reshape(128, 1)   # mmC path lacks A0^T b3
    b1_boot = b1.reshape(128, 1)
    b2v = b2.reshape(128, 1)
    b3v = b3.reshape(8, 1)
    ident = np.eye(128, dtype=np.float16)

    in_maps = []
    for c in range(NCORES):
        bs = slice(c * BC, (c + 1) * BC)
        u_c = useq[bs]      # [BC, T, 4]
        yz_c = yz0[bs]      # [BC, 56]

        stag0 = np.zeros((128, BC), dtype=np.float32)
        stag0[0:8] = yz_c[:, 0:8].T               # slot 0 = y_0
        for s in range(4):                         # boot blocks y_{-(s+1)}
            blk = yz_c[:, 8 + 8 * (3 - s):16 + 8 * (3 - s)]  # ypseq newest last
            stag0[32 * s + 8:32 * s + 16] = blk.T

        # sliding u-windows for the K=20 u matmul
        uhist = yz_c[:, 40:56].reshape(BC, 4, 4)          # u_{-4..-1}
        uext = np.concatenate([uhist, u_c], axis=1)       # [BC, T+4, 4]
        sw = np.lib.stride_tricks.sliding_window_view(uext, 5, axis=1)
        # sw: [BC, T, 4, 5] -> uwin [T, 20, BC] (chronological rows)
        uwin = np.ascontiguousarray(sw.transpose(1, 3, 2, 0).reshape(T, 20, BC))

        in_maps.append({
            "stag0": stag0.astype(np.float16),
            "uwin": uwin.astype(np.float16),
            "bs": np.ascontiguousarray(Bstack).astype(np.float16),
            "zcf": np.zeros((128, BC), dtype=np.float16),
            "cmats": cmats2d.astype(np.float16),
            "w2": W2.astype(np.float16),
            "wc": WC.astype(np.float16),
            "w3": W3.astype(np.float16),
            "b1v": np.ascontiguousarray(b1_eff),
            "b1b": np.ascontiguousarray(b1_boot),
            "b2v": np.ascontiguousarray(b2v),
            "b3v": np.ascontiguousarray(b3v),
        })
    return in_maps


def get_program(zero_bias=False):
    key = ("nc", bool(zero_bias))
    if key not in _COMPILED:
        _COMPILED[key] = _build_program(zero_bias)
    return _COMPILED[key]


def _enable_ldw_opt():
    """Allow walrus to double-buffer LDWEIGHTS (background weight loads).

    The environment default is --enable-ldw-opt=false, which serializes
    every LDWEIGHTS behind the previous matmul's drain; with ~6 weight
    switches per RNN step that costs ~2x on the tensor engine.
    """
    try:
        from concourse.compiler_utils import get_compiler_flags, set_compiler_flags

        flags = get_compiler_flags()
        new = [f.replace("--enable-ldw-opt=false", "--enable-ldw-opt=true") for f in flags]
        if new != flags:
            set_compiler_flags(new)
    except Exception:
        pass


def run_cores(in_maps, zero_bias=False, **kwargs):
    from concourse.bass_utils import run_bass_kernel_spmd

    _enable_ldw_opt()
    nc = get_program(zero_bias)
    return run_bass_kernel_spmd(nc, in_maps, core_ids=list(range(NCORES)), **kwargs)


def assemble(res):
    outs = []
    for r in res.results:
        buf = np.asarray(r["out2"], dtype=np.float32)   # [T/4, 4, 8, BC]
        ys = buf.transpose(3, 0, 1, 2).reshape(BC, T, NY)
        outs.append(ys)
    return np.concatenate(outs, axis=0)


def kernel(useq, yz0, W1, b1, W2, b2, W3, b3):
    in_maps = _host_prep(useq, yz0, W1, b1, W2, b2, W3, b3)
    zb = bool(
        np.all(in_maps[0]["b1v"] == 0.0)
        and np.all(in_maps[0]["b1b"] == 0.0)
        and np.all(in_maps[0]["b2v"] == 0.0)
    )
    res = run_cores(in_maps, zero_bias=zb)
    return assemble(res)



# revision 27
# speedup vs baseline: 1.8723x; 1.0093x over previous
"""Trainium2 Bass kernel for the NP/NY/NU RNN scan (nn_BlackBoxModel_24489903521937).

Model (per step t, batch row b):
    x_t   = [y_t, y_{t-4..t-1}, u_{t-4..t-1}, u_t]          (60)
    h1    = tanh(x_t @ W1 + b1)                              (128)
    h2    = tanh(h1 @ W2 + b2)                               (128)
    y_{t+1} = h2 @ W3 + b3                                   (8)
    output ys[:, t] = y_t

Strategy (pure data parallel, batch 4096 -> 8 cores x 512):
  * feature-major layout: features on SBUF partitions, batch on the free dim.
  * y-history lives in 4 ring slots of a [128, B] staging tile, one slot per
    32-partition strip (SBUF APs must start at partition 0/32/64/96).  The
    x @ W1 product becomes: one K=128 matmul against phase-permuted W1 blocks
    (C_p, zero rows where a slot is semantically dead), one K=20 sliding
    u-window matmul, and one composed (W3 @ A0) matmul from h2 directly, so
    the recurrent cycle is just tanh -> mm(W2) -> tanh -> mm(W3 A0).
  * y_{t-4} is read from the slot y_t is about to overwrite: emission order
    (mmX before the staging write) makes Tile sequence the write after the
    read, so no extra buffering is needed.
  * outputs retire from the staging tile by raw feature-major DMA every 4
    steps; the host does the final [T,8,B] -> [B,T,8] transpose.
  * matmul operands are fp16 (1 cycle/row, fp32 PSUM accumulate); the
    5-step fading memory of the state keeps fp16 error flat (~6e-4).
"""

import numpy as np

NP_, NY, NU = 4, 8, 4
B, T, H = 4096, 256, 128
NCORES = 8
BC = B // NCORES  # 512 batch rows per core
CHUNKS = 2        # column chunks for the critical tanh/matmul cycle
CW = BC // CHUNKS
PF = 6            # u-window DMA prefetch depth (steps ahead)
NSLOT = 4         # y ring slots (one per 32-partition strip)

_COMPILED = {}


def _build_program(zero_bias):
    import concourse.mybir as mybir
    import concourse.tile as tile
    from concourse import bacc

    f32 = mybir.dt.float32
    fh = mybir.dt.float16
    Tanh = mybir.ActivationFunctionType.Tanh

    nc = bacc.Bacc("TRN2", target_bir_lowering=False, debug=False)

    d_stag0 = nc.dram_tensor("stag0", [128, BC], fh, kind="ExternalInput")
    d_uwin = nc.dram_tensor("uwin", [T, 20, BC], fh, kind="ExternalInput")
    d_bs = nc.dram_tensor("bs", [20, 128], fh, kind="ExternalInput")
    # packed blob: 8 C matrices (phases 0..3 + boot 0..3), then W2, WC, W3
    d_wblob = nc.dram_tensor("wblob", [128, 8 * 128 + 264], fh, kind="ExternalInput")
    if not zero_bias:
        d_b1 = nc.dram_tensor("b1v", [128, 1], f32, kind="ExternalInput")
        d_b1b = nc.dram_tensor("b1b", [128, 1], f32, kind="ExternalInput")
        d_b2 = nc.dram_tensor("b2v", [128, 1], f32, kind="ExternalInput")
    d_b3 = nc.dram_tensor("b3v", [8, 1], f32, kind="ExternalInput")
    d_out2 = nc.dram_tensor("out2", [T // 4, 4, 8, BC], fh, kind="ExternalOutput")
    d_warm = nc.dram_tensor("warm", [8, 16], fh, kind="ExternalOutput")

    with tile.TileContext(nc) as tc:
        with (
            tc.tile_pool(name="const", bufs=1) as cpool,
            tc.tile_pool(name="stagp", bufs=1) as spool,
            tc.tile_pool(name="upool", bufs=8) as upool,
            tc.tile_pool(name="hpool", bufs=2) as hpool,
            tc.tile_pool(name="ph1a", bufs=2, space="PSUM") as ph1ap,
            tc.tile_pool(name="ph1b", bufs=2, space="PSUM") as ph1bp,
            tc.tile_pool(name="ph2a", bufs=1, space="PSUM") as ph2ap,
            tc.tile_pool(name="ph2b", bufs=1, space="PSUM") as ph2bp,
            tc.tile_pool(name="pypa", bufs=1, space="PSUM") as pypap,
            tc.tile_pool(name="pypb", bufs=1, space="PSUM") as pypbp,
        ):
            t_wb = cpool.tile_from(d_wblob[:])
            t_cm = t_wb[:, 0:1024]
            t_w2 = t_wb[:, 1024:1152]
            t_wc = t_wb[:, 1152:1280]
            t_w3 = t_wb[:, 1280:1288]
            t_bs = cpool.tile_from(d_bs[:])
            if not zero_bias:
                t_b1 = cpool.tile_from(d_b1[:])
                t_b1b = cpool.tile_from(d_b1b[:])
                t_b2 = cpool.tile_from(d_b2[:])
            t_b3 = cpool.tile_from(d_b3[:])
            # scratch operands for the PE warm-up so it can run concurrently
            # with the input DMAs (values are discarded); gpsimd memset keeps
            # the tile allocator happy without touching the DMA queues
            t_scr = cpool.tile([128, BC], fh, name="scr")
            nc.gpsimd.memset(t_scr[:, :], 0.0)

            stag = spool.tile([128, BC], fh, name="stag")
            nc.sync.dma_start(stag[:], d_stag0[:])

            # --- PE clock warm-up: ~6us of back-to-back matmuls trips the
            # HAM clock gate from 1.2 GHz (cold K=4/8) to 2.4 GHz before the
            # recurrence starts.  Results land in a scratch PSUM tile whose
            # corner is exported so the chain stays live.
            warm_p = ph2ap.tile([128, BC], f32, name="warmp", tag="h2pa")
            for _ in range(10):
                nc.tensor.matmul(
                    warm_p[:, :], t_scr[:, 0:128], t_scr[:, :],
                    start=True, stop=True, skip_group_check=True,
                )
            warm_s = cpool.tile([8, 16], fh, name="warms")
            nc.scalar.copy(warm_s[:, :], warm_p[0:8, 0:16])
            nc.sync.dma_start(d_warm[:], warm_s[:, :])

            def cmat(i):
                return t_wb[:, 128 * i:128 * i + 128]

            utiles = {}

            def prefetch_u(tt):
                ut = upool.tile([20, BC], fh, name="uw", tag="uw")
                nc.sync.dma_start(ut[:], d_uwin[tt])
                utiles[tt] = ut

            for tt in range(PF):
                prefetch_u(tt)

            CA = slice(0, CW)
            CB = slice(CW, BC)

            def emit_group_xu(tt, phA, phB):
                """Open both half groups for step tt: per half (own PSUM
                bank) mmU (start=True, the bank-wide has_written clear) then
                the y-history matmul.  Each half group is closed later by its
                own mmC, so tanh1 of one half never waits on the other."""
                cidx = 4 + tt if tt < 4 else tt % NSLOT
                ut = utiles.pop(tt)
                for ph, cs in ((phA, CA), (phB, CB)):
                    nc.tensor.matmul(
                        ph[:, :],
                        t_bs[:, :],
                        ut[:, cs],
                        start=True, stop=False, skip_group_check=True,
                    )
                for ph, cs in ((phA, CA), (phB, CB)):
                    nc.tensor.matmul(
                        ph[:, :],
                        cmat(cidx),
                        stag[:, cs],
                        start=False, stop=(tt == 0), skip_group_check=True,
                    )

            def flush(ty):
                """Export y_{ty} (slot ty%4) feature-major to DRAM; the host
                transposes to batch-major at the end."""
                s = ty % 4
                nc.sync.dma_start(d_out2[ty // 4, s], stag[32 * s:32 * s + 8, :])

            phA_cur = ph1ap.tile([128, CW], f32, name="h1pa", tag="h1pa")
            phB_cur = ph1bp.tile([128, CW], f32, name="h1pb", tag="h1pb")
            emit_group_xu(0, phA_cur, phB_cur)

            for t in range(T):
                if zero_bias:
                    bias1 = bias2 = 0.0
                else:
                    b1t = t_b1b if t == 0 else t_b1
                    bias1 = b1t[:, 0:1]
                    bias2 = t_b2[:, 0:1]

                # --- tanh1 chunks 1a, 1b ---
                h1_t = hpool.tile([128, BC], fh, name="h1", tag="h1")
                nc.scalar.activation(
                    h1_t[:, CA], phA_cur[:, :], Tanh, bias=bias1
                )
                nc.scalar.activation(
                    h1_t[:, CB], phB_cur[:, :], Tanh, bias=bias1
                )

                # --- mm2 per half (separate PSUM banks so tanh2a only
                #     depends on mm2a) ---
                ph2a_t = ph2ap.tile([128, CW], f32, name="h2pa", tag="h2pa")
                ph2b_t = ph2bp.tile([128, CW], f32, name="h2pb", tag="h2pb")
                nc.tensor.matmul(ph2a_t[:, :], t_w2[:, :], h1_t[:, CA])
                nc.tensor.matmul(ph2b_t[:, :], t_w2[:, :], h1_t[:, CB])

                # --- open next step's half groups (off the critical chain;
                #     emitted before this step's staging writes so the stale
                #     y_{t-3} slot read stays dependency-free) ---
                phA_next = phB_next = None
                if t + 1 < T:
                    phA_next = ph1ap.tile([128, CW], f32, name="h1pa", tag="h1pa")
                    phB_next = ph1bp.tile([128, CW], f32, name="h1pb", tag="h1pb")
                    emit_group_xu(t + 1, phA_next, phB_next)

                # --- tanh2 chunks 2a, 2b ---
                h2_t = hpool.tile([128, BC], fh, name="h2", tag="h2")
                nc.scalar.activation(
                    h2_t[:, CA], ph2a_t[:, :], Tanh, bias=bias2
                )
                nc.scalar.activation(
                    h2_t[:, CB], ph2b_t[:, :], Tanh, bias=bias2
                )

                # --- output flush (1 slot/step; ~3 steps of slack) ---
                if t >= 1:
                    flush(t - 1)

                # --- close the half groups + y_{t+1} = W3^T h2 + b3:
                #     PE order mmCa, mm3a, mm3b, mmCb ---
                if t + 1 < T:
                    pypa_t = pypap.tile([8, CW], f32, name="ypa", tag="ypa")
                    pypb_t = pypbp.tile([8, CW], f32, name="ypb", tag="ypb")
                    nc.tensor.matmul(
                        phA_next[:, :], t_wc[:, :], h2_t[:, CA],
                        start=False, stop=True, skip_group_check=True,
                    )
                    nc.tensor.matmul(pypa_t[:, :], t_w3[:, :], h2_t[:, CA])
                    nc.tensor.matmul(
                        phB_next[:, :], t_wc[:, :], h2_t[:, CB],
                        start=False, stop=True, skip_group_check=True,
                    )
                    nc.tensor.matmul(pypb_t[:, :], t_w3[:, :], h2_t[:, CB])
                    p0 = 32 * ((t + 1) % NSLOT)
                    nc.vector.tensor_scalar_add(
                        stag[p0:p0 + 8, CA], pypa_t[:, :], t_b3[:, 0:1]
                    )
                    nc.vector.tensor_scalar_add(
                        stag[p0:p0 + 8, CB], pypb_t[:, :], t_b3[:, 0:1]
                    )

                if t + PF < T:
                    prefetch_u(t + PF)

                phA_cur = phA_next
                phB_cur = phB_next

            flush(T - 1)

    nc.compile()
    return nc


def _host_prep(useq, yz0, W1, b1, W2, b2, W3, b3):
    """Build the per-core input maps (all host-side numpy)."""
    useq = np.ascontiguousarray(useq, dtype=np.float32)
    yz0 = np.ascontiguousarray(yz0, dtype=np.float32)
    W1 = np.asarray(W1, dtype=np.float32)
    W2 = np.ascontiguousarray(W2, dtype=np.float32)
    W3 = np.ascontiguousarray(W3, dtype=np.float32)
    b1 = np.asarray(b1, dtype=np.float32)
    b2 = np.asarray(b2, dtype=np.float32)
    b3 = np.asarray(b3, dtype=np.float32)

    A = {0: W1[0:8], 4: W1[8:16], 3: W1[16:24], 2: W1[24:32], 1: W1[32:40]}
    Bstack = W1[40:60]  # u_{t-4..t} stacked chronologically

    # staging rows: slot s -> [32s, 32s+8) holds y ring;
    #               boot block s -> [32s+8, 32s+16) holds y_{-(s+1)}
    cmats = np.zeros((8, 128, 128), dtype=np.float32)
    for p in range(NSLOT):  # steady phases, t >= 4: every slot one A_k
        for s in range(NSLOT):
            k = ((p - s - 1) % 4) + 1
            cmats[p, 32 * s:32 * s + 8] = A[k]
    for tt in range(4):  # boot steps t=0..3
        cb = cmats[4 + tt]
        for k in range(1, 5):
            if tt - k >= 0:
                s = (tt - k) % 4
                cb[32 * s:32 * s + 8] += A[k]
            else:
                s = k - tt - 1
                cb[32 * s + 8:32 * s + 16] += A[k]
        if tt == 0:
            cb[0:8] += A[0]  # slot 0 carries y_0 directly at t=0
    cmats2d = np.ascontiguousarray(
        cmats.transpose(1, 0, 2).reshape(128, 8 * 128)
    )

    WC = np.ascontiguousarray(W3 @ A[0])          # [128, 128]
    b1_eff = (b1 + A[0].T @ b3).reshape(128, 1)   # mmC path lacks A0^T b3
    b1_boot = b1.reshape(128, 1)
    b2v = b2.reshape(128, 1)
    b3v = b3.reshape(8, 1)
    ident = np.eye(128, dtype=np.float16)

    in_maps = []
    for c in range(NCORES):
        bs = slice(c * BC, (c + 1) * BC)
        u_c = useq[bs]      # [BC, T, 4]
        yz_c = yz0[bs]      # [BC, 56]

        stag0 = np.zeros((128, BC), dtype=np.float32)
        stag0[0:8] = yz_c[:, 0:8].T               # slot 0 = y_0
        for s in range(4):                         # boot blocks y_{-(s+1)}
            blk = yz_c[:, 8 + 8 * (3 - s):16 + 8 * (3 - s)]  # ypseq newest last
            stag0[32 * s + 8:32 * s + 16] = blk.T

        # sliding u-windows for the K=20 u matmul
        uhist = yz_c[:, 40:56].reshape(BC, 4, 4)          # u_{-4..-1}
        uext = np.concatenate([uhist, u_c], axis=1)       # [BC, T+4, 4]
        sw = np.lib.stride_tricks.sliding_window_view(uext, 5, axis=1)
        # sw: [BC, T, 4, 5] -> uwin [T, 20, BC] (chronological rows)
        uwin = np.ascontiguousarray(sw.transpose(1, 3, 2, 0).reshape(T, 20, BC))

        wblob = np.concatenate([cmats2d, W2, WC, W3], axis=1)
        in_maps.append({
            "stag0": stag0.astype(np.float16),
            "uwin": uwin.astype(np.float16),
            "bs": np.ascontiguousarray(Bstack).astype(np.float16),
            "wblob": np.ascontiguousarray(wblob).astype(np.float16),
            "b1v": np.ascontiguousarray(b1_eff),
            "b1b": np.ascontiguousarray(b1_boot),
            "b2v": np.ascontiguousarray(b2v),
            "b3v": np.ascontiguousarray(b3v),
        })
    return in_maps


def get_program(zero_bias=False):
    key = ("nc", bool(zero_bias))
    if key not in _COMPILED:
        _COMPILED[key] = _build_program(zero_bias)
    return _COMPILED[key]


def _enable_ldw_opt():
    """Allow walrus to double-buffer LDWEIGHTS (background weight loads).

    The environment default is --enable-ldw-opt=false, which serializes
    every LDWEIGHTS behind the previous matmul's drain; with ~6 weight
    switches per RNN step that costs ~2x on the tensor engine.
    """
    try:
        from concourse.compiler_utils import get_compiler_flags, set_compiler_flags

        flags = get_compiler_flags()
        new = [f.replace("--enable-ldw-opt=false", "--enable-ldw-opt=true") for f in flags]
        if new != flags:
            set_compiler_flags(new)
    except Exception:
        pass


def run_cores(in_maps, zero_bias=False, **kwargs):
    from concourse.bass_utils import run_bass_kernel_spmd

    _enable_ldw_opt()
    nc = get_program(zero_bias)
    if zero_bias:
        drop = {"b1v", "b1b", "b2v"}
        in_maps = [{k: v for k, v in m.items() if k not in drop} for m in in_maps]
    return run_bass_kernel_spmd(nc, in_maps, core_ids=list(range(NCORES)), **kwargs)


def assemble(res):
    outs = []
    for r in res.results:
        buf = np.asarray(r["out2"], dtype=np.float32)   # [T/4, 4, 8, BC]
        ys = buf.transpose(3, 0, 1, 2).reshape(BC, T, NY)
        outs.append(ys)
    return np.concatenate(outs, axis=0)


def kernel(useq, yz0, W1, b1, W2, b2, W3, b3):
    in_maps = _host_prep(useq, yz0, W1, b1, W2, b2, W3, b3)
    zb = bool(
        np.all(in_maps[0]["b1v"] == 0.0)
        and np.all(in_maps[0]["b1b"] == 0.0)
        and np.all(in_maps[0]["b2v"] == 0.0)
    )
    res = run_cores(in_maps, zero_bias=zb)
    return assemble(res)

